# revision 16
# baseline (speedup 1.0000x reference)
"""Trainium2 Bass kernel for nn_CorrTorch: 27-shift 3D correlation + 1x1x1 conv.

Math (B=1, C=32, D=H=W=64, NOFF=27):
  cv[(k,c), s] = x1[c,s] * pad(x2)[c, s + off_k] / sqrt(C)    (864 x 64^3)
  out[o, s]    = sum_{k,c} conv_w[o, k*32+c] * cv[(k,c), s] + conv_b[o]

Sharding: D axis split across 8 cores (8 planes each), halo baked into the
per-core x2 slab on the host. No collectives.

Per-core device strategy (one output plane d at a time):
  - x2 slab planes carry FOUR replica groups of 32 channels on 128 partitions;
    group g is pre-shifted (host-baked) by o_g in (dy,dx):
        O = [(0,0), (-2,0), (-1,-1), (-1,1)]
    With a per-instruction 2D slice offset s, one tensor_tensor multiply
    computes the 4 products {o_g + s} at once.  Exhaustive search over all
    4-offset bakes shows 9 multiplies/plane is the floor (27 shifts, <=4 new
    per instruction, and no 4-cell pattern has >3 disjoint in-cube translates);
    this bake achieves it as 2 fulls + 1 half-width partial per dz:
        dz in {0,2}:  full@s=(2,0), full@s=(2,1), groups{0,1}@s=(2,2)
        dz == 1:      full@s=(1,1), full@s=(2,1), groups{2,3}@s=(3,1)
    covering each of the 27 cells exactly once.
  - The two complementary partials of dz=0/dz=1 stack into ONE 128-row cv
    tile, so the 9 products form only EIGHT matmul contraction chunks
    (7x K=128 + 1x K=64) instead of 9 -> tensor-engine time drops ~11%.
  - ~20 of the 72 multiplies run on GPSIMD via scalar_tensor_tensor
    ((x*1)*y), which maps to a faster Q7 codepath than tensor_tensor mult;
    the rest run on the (bottleneck) vector engine in bf16 2x mode.
  - The 1x1 conv is 8 accumulated matmuls per 512-col subtile with M=32
    (27 outputs + 5 zero rows so PSUM is fully written), 4 spatial subtiles
    packed per PSUM bank via tile_position column tiling.
  - ScalarE evicts PSUM -> SBUF bf16 with the conv bias applied, one DMA per
    PSUM tile writes a packed [128, 512] block to HBM; the host un-packs.
"""

import numpy as np
import ml_dtypes

import concourse.bass as bass
import concourse.mybir as mybir
import concourse.tile as tile
from concourse.alu_op_type import AluOpType
from concourse.bass_utils import run_bass_kernel_spmd

C = 32
D = 64
H = 64
W = 64
NOFF = 27
NCORES = 8
DLOC = D // NCORES          # 8 output planes per core
NSLAB = DLOC + 2            # 10 slab planes per core
SROWS = 67                  # baked slab rows per plane (dyv in [0,67))
SCOLS = 66                  # baked slab cols per plane
PLANE_F = SROWS * SCOLS     # 4422 elements per partition per slab plane
TN = H * W                  # 4096 columns per cv tile (one full plane)
SUB = 512                   # columns per matmul (one PSUM bank)
NSUBT = TN // SUB           # 8 spatial subtiles per plane
NCHUNK = 8                  # matmul contraction chunks per plane
M32 = 32                    # matmul output rows per col group (27 + 5 zero)

# Replica-group pre-shift offsets (dy, dx), searched (see module docstring).
OFFS = [(0, 0), (-2, 0), (-1, -1), (-1, 1)]

# Per-plane chunk specs: (dz, s=(sy,sx), partition ranges with group lists).
# Every chunk is one cv tile; chunk 6 holds two half-width instructions.
# cells(g) = OFFS[g] + s must cover {0,1,2}^2 per dz exactly (checked below).
CHUNKS = [
    (0, [((0, 128), (2, 0))]),
    (0, [((0, 128), (2, 1))]),
    (1, [((0, 128), (1, 1))]),
    (1, [((0, 128), (2, 1))]),
    (2, [((0, 128), (2, 0))]),
    (2, [((0, 128), (2, 1))]),
    (None, [((0, 64), (2, 2), 0), ((64, 128), (3, 1), 1)]),  # dz 0 / dz 1
    (2, [((0, 64), (2, 2))]),
]
CHUNK_K = [128, 128, 128, 128, 128, 128, 128, 64]

# Engine balance: DVE does bf16 tensor_tensor at ~0.52 ns/col; GPSIMD does the
# same multiply ~2.7x slower but in parallel.  Per plane, GPSIMD takes chunks
# 7 and 6[0:64] whole plus the tail columns of 6[64:128]; DVE takes the rest.
XSPLIT = 2240  # columns of chunk 6[64:128] on DVE (row-aligned); rest GPSIMD

# Matmul accumulation order: GPSIMD-produced tiles first (they have the
# longest latency but are issued a plane ahead), DVE tiles in issue order.
MM_ORDER = [7, 6, 0, 1, 2, 3, 4, 5]

BF16 = mybir.dt.bfloat16
F32 = mybir.dt.float32

_wsplit_ctr = [0]


def _split_sync_waits(nc, max_waits=1):
    """Walrus in this container accepts at most one sync wait per instruction.
    Hoist excess waits onto NoOp instructions inserted just before, on the
    same engine (same-engine program order preserves the semantics)."""
    for fn in nc.m.functions:
        for bb in fn.blocks:
            new = []
            changed = False
            for ins in bb.instructions:
                si = ins.sync_info
                if si is not None and len(si.on_wait) > max_waits:
                    waits = list(si.on_wait)
                    excess, keep = waits[:-max_waits], waits[-max_waits:]
                    for i in range(0, len(excess), max_waits):
                        _wsplit_ctr[0] += 1
                        new.append(
                            mybir.InstNoOp(
                                name=f"wsplit-{_wsplit_ctr[0]}",
                                engine=ins.engine,
                                sync_info=mybir.SyncInfo(
                                    on_wait=excess[i : i + max_waits], on_update=[]
                                ),
                            )
                        )
                    ins.sync_info = mybir.SyncInfo(
                        on_wait=keep, on_update=list(si.on_update)
                    )
                    changed = True
                new.append(ins)
            if changed:
                bb.instructions = new


def _chunk_cells():
    """Resolve CHUNKS into per-chunk (dz, cell, group) lists and verify the
    27-cell cover is exact."""
    out = []
    seen = set()
    for ci, (dz0, instrs) in enumerate(CHUNKS):
        cells = []
        for spec in instrs:
            (p0, p1), (sy, sx) = spec[0], spec[1]
            dz = dz0 if len(spec) == 2 else spec[2]
            for g in range(p0 // 32, p1 // 32):
                oy, ox = OFFS[g]
                dy, dx = sy + oy, sx + ox
                if 0 <= dy <= 2 and 0 <= dx <= 2:
                    cell = (dz, dy, dx)
                    assert cell not in seen, (ci, cell)
                    seen.add(cell)
                    cells.append((g, dz, dy, dx))
        out.append(cells)
    assert len(seen) == 27, len(seen)
    return out


_CELLS = _chunk_cells()


def build_program():
    nc = bass.Bass()

    x1r = nc.dram_tensor("x1r", [DLOC, 128, TN], BF16, kind="ExternalInput")
    x2r = nc.dram_tensor("x2r", [NSLAB, 128, PLANE_F], BF16, kind="ExternalInput")
    wts = nc.dram_tensor("wts", [128, NCHUNK * M32], BF16, kind="ExternalInput")
    bias = nc.dram_tensor("bias", [128, 1], F32, kind="ExternalInput")
    out = nc.dram_tensor("out", [128, DLOC * 2 * SUB], BF16, kind="ExternalOutput")

    with tile.TileContext(nc) as tc:
        with (
            tc.tile_pool(name="wt", bufs=1) as wt_pool,
            tc.tile_pool(name="x2", bufs=5) as x2_pool,
            tc.tile_pool(name="x1", bufs=3) as x1_pool,
            tc.tile_pool(name="cv", bufs=16) as cv_pool,
            tc.tile_pool(name="stage", bufs=3) as stage_pool,
            tc.tile_pool(name="psum", bufs=6, space="PSUM") as psum_pool,
        ):
            x2t = {}

            def load_x2_plane(p):
                t = x2_pool.tile([128, PLANE_F], BF16, tag="x2plane")
                nc.sync.dma_start(out=t[:], in_=x2r[p])
                x2t[p] = t

            # Pipeline fill: weights first (tiny), then the pieces that the
            # scheduler's first few DVE/PE instructions depend on, smallest
            # dependencies first so compute starts ~6us in.
            wt_tile = wt_pool.tile([128, NCHUNK * M32], BF16)
            nc.sync.dma_start(out=wt_tile[:], in_=wts[:])
            bias_tile = wt_pool.tile([128, 1], F32)
            nc.sync.dma_start(out=bias_tile[:], in_=bias[:])

            t1 = x2_pool.tile([128, PLANE_F], BF16, tag="x2plane")
            nc.sync.dma_start(out=t1[:, 0:2508], in_=x2r[1][:, 0:2508])
            x2t[1] = t1
            x1t0 = x1_pool.tile([128, TN], BF16, tag="x1plane")
            nc.sync.dma_start(out=x1t0[:, 0:2304], in_=x1r[0][:, 0:2304])
            t0 = x2_pool.tile([128, PLANE_F], BF16, tag="x2plane")
            nc.sync.dma_start(out=t0[:, 0:2310], in_=x2r[0][:, 0:2310])
            x2t[0] = t0
            nc.sync.dma_start(out=t0[:, 2310:], in_=x2r[0][:, 2310:])
            nc.sync.dma_start(out=x1t0[:, 2304:], in_=x1r[0][:, 2304:])
            nc.sync.dma_start(out=t1[:, 2508:], in_=x2r[1][:, 2508:])
            load_x2_plane(2)

            for d in range(DLOC):
                if d + 3 < NSLAB:
                    load_x2_plane(d + 3)
                if d == 0:
                    x1t = x1t0
                else:
                    x1t = x1_pool.tile([128, TN], BF16, tag="x1plane")
                    nc.sync.dma_start(out=x1t[:], in_=x1r[d])

                # --- 9 multiplies (split 9.5/2.5-ish DVE/GPSIMD) -> 8 cv tiles
                cvt = []
                for _ci in range(NCHUNK):
                    cv = cv_pool.tile([128, TN], BF16, tag="cv")
                    cvt.append(cv)

                def mult(eng, cv, dz, p0, p1, sy, sx, c0, c1):
                    # c0/c1 must be multiples of W (row-aligned column split)
                    y0, y1 = sy + c0 // W, sy + c1 // W
                    slab = x2t[d + dz][:].rearrange(
                        "p (y x) -> p y x", y=SROWS, x=SCOLS
                    )
                    in1 = slab[p0:p1, y0:y1, sx : sx + W]
                    if eng == "pool":
                        nc.gpsimd.scalar_tensor_tensor(
                            out=cv[p0:p1, c0:c1],
                            in0=x1t[p0:p1, c0:c1],
                            scalar=1.0,
                            in1=in1,
                            op0=AluOpType.mult,
                            op1=AluOpType.mult,
                        )
                    else:
                        nc.vector.tensor_mul(
                            out=cv[p0:p1, c0:c1], in0=x1t[p0:p1, c0:c1], in1=in1
                        )

                def full(ci):
                    dz0, instrs = CHUNKS[ci]
                    (p0, p1), (sy, sx) = instrs[0][0], instrs[0][1]
                    mult("dve", cvt[ci], dz0, p0, p1, sy, sx, 0, TN)

                if d == 0:
                    # Pipeline fill: the scheduler runs the chunk-6 suffix
                    # head first on DVE; its (split) loads land first, and the
                    # first full multiply is halved to follow the partials.
                    mult("pool", cvt[6], 0, 0, 64, 2, 2, 0, TN)
                    mult("pool", cvt[7], 2, 0, 64, 2, 2, 0, TN)
                    mult("pool", cvt[6], 1, 64, 128, 3, 1, XSPLIT, TN)
                    mult("dve", cvt[6], 1, 64, 128, 3, 1, 0, XSPLIT)
                    mult("dve", cvt[0], 0, 0, 128, 2, 0, 0, 2048)
                    mult("dve", cvt[0], 0, 0, 128, 2, 0, 2048, TN)
                    for ci in (1, 2, 3, 4, 5):
                        full(ci)
                else:
                    # GPSIMD: chunk 7, chunk 6 prefix, tail cols of 6 suffix
                    mult("pool", cvt[7], 2, 0, 64, 2, 2, 0, TN)
                    mult("pool", cvt[6], 0, 0, 64, 2, 2, 0, TN)
                    mult("pool", cvt[6], 1, 64, 128, 3, 1, XSPLIT, TN)
                    # DVE: head cols of chunk 6 suffix, then the 6 full chunks
                    mult("dve", cvt[6], 1, 64, 128, 3, 1, 0, XSPLIT)
                    for ci in range(6):
                        full(ci)

                # --- 8 accumulated matmul chunks, 2 PSUM tiles x 4 col groups
                psums = []
                for _ph in range(2):
                    ps = psum_pool.tile([128, SUB], F32, tag="ps")
                    psums.append(ps)
                order = list(range(NCHUNK)) if d == 0 else MM_ORDER
                for mi, ci in enumerate(order):
                    kp = CHUNK_K[ci]
                    for s in range(NSUBT):
                        nc.tensor.matmul(
                            psums[s // 4][32 * (s % 4) : 32 * (s % 4) + M32, :],
                            lhsT=wt_tile[0:kp, ci * M32 : (ci + 1) * M32],
                            rhs=cvt[ci][0:kp, s * SUB : (s + 1) * SUB],
                            start=(mi == 0),
                            stop=(mi == NCHUNK - 1),
                            tile_position=(0, 32 * (s % 4)),
                        )

                for half in range(2):
                    stage = stage_pool.tile([128, SUB], BF16, tag="stage")
                    nc.scalar.activation(
                        stage[:],
                        psums[half][:],
                        mybir.ActivationFunctionType.Identity,
                        bias=bias_tile[:],
                    )
                    col = (2 * d + half) * SUB
                    nc.sync.dma_start(
                        out=out[:, col : col + SUB], in_=stage[:]
                    )

    _split_sync_waits(nc)
    return nc


_PROGRAM = None


def _get_program():
    global _PROGRAM
    if _PROGRAM is None:
        _PROGRAM = build_program()
    return _PROGRAM


def _prep_inputs(in1, in2, conv_w, conv_b):
    """Build the 8 per-core input maps (bf16 layout prep on host)."""
    x1 = np.ascontiguousarray(np.asarray(in1, np.float32).reshape(C, D, H, W))
    x2 = np.ascontiguousarray(np.asarray(in2, np.float32).reshape(C, D, H, W))
    scale = 1.0 / np.sqrt(np.float32(C))
    Wk = (np.asarray(conv_w, np.float32) * scale).reshape(NOFF, NOFF, C)  # [o,k,c]

    # Weights: [128, 8*32]; row 32g+c, col 32*ci+o = Wk[o, 9dz+3dy+dx, c]
    wts = np.zeros((128, NCHUNK * M32), np.float32)
    for ci, cells in enumerate(_CELLS):
        for (g, dz, dy, dx) in cells:
            k = 9 * dz + 3 * dy + dx
            wts[32 * g : 32 * g + C, ci * M32 : ci * M32 + NOFF] = Wk[:, k, :].T
    wts = wts.astype(ml_dtypes.bfloat16)

    bias128 = np.zeros((128, 1), np.float32)
    cb = np.asarray(conv_b, np.float32)
    for g4 in range(4):
        bias128[32 * g4 : 32 * g4 + NOFF, 0] = cb

    # Globally padded x2 volume: [C, D+2, 66, 66]; plane/row/col = global + 1.
    x2p = np.zeros((C, D + 2, H + 2, W + 2), np.float32)
    x2p[:, 1 : D + 1, 1 : H + 1, 1 : W + 1] = x2

    # Baked slab: buffer_g[dyv, dxv] = x2p_plane[dyv + oy, dxv + ox] (0 outside)
    # so an instruction slice (sy, sx) reads x2p_plane[y + sy + oy, x + sx + ox].
    # Widened scratch plane: rows -2..66 -> 69 (offset +2), cols -1..66 -> 68 (+1).
    in_maps = []
    for m in range(NCORES):
        slab = x2p[:, DLOC * m : DLOC * m + NSLAB]  # [C, 10, 66, 66]
        wide = np.zeros((C, NSLAB, 69, 68), np.float32)
        wide[:, :, 2:68, 1:67] = slab
        x2rep = np.zeros((NSLAB, 128, SROWS, SCOLS), np.float32)
        for g, (oy, ox) in enumerate(OFFS):
            x2rep[:, 32 * g : 32 * g + C] = wide[
                :, :, 2 + oy : 2 + oy + SROWS, 1 + ox : 1 + ox + SCOLS
            ].transpose(1, 0, 2, 3)
        x2rep = x2rep.reshape(NSLAB, 128, PLANE_F).astype(ml_dtypes.bfloat16)

        x1c = x1[:, DLOC * m : DLOC * (m + 1)].reshape(C, DLOC, TN)
        x1rep = (
            np.tile(x1c, (4, 1, 1))
            .reshape(128, DLOC, TN)
            .transpose(1, 0, 2)
            .astype(ml_dtypes.bfloat16)
        )  # [8, 128, 4096]

        in_maps.append(
            {
                "x1r": np.ascontiguousarray(x1rep),
                "x2r": np.ascontiguousarray(x2rep),
                "wts": np.ascontiguousarray(wts),
                "bias": bias128,
            }
        )
    return in_maps


def kernel(in1, in2, conv_w, conv_b):
    nc = _get_program()
    in_maps = _prep_inputs(in1, in2, conv_w, conv_b)
    res = run_bass_kernel_spmd(nc, in_maps, core_ids=list(range(NCORES)))
    outs = []
    for r in res.results:
        # [128, DLOC*2*512] bf16: row 32*g4+o, col (2d+half)*512+c
        a = np.asarray(r["out"], np.float32).reshape(4, 32, DLOC, 2, SUB)
        # -> [o, d, half, g4, c] -> [o, d, 4096]
        core = a.transpose(1, 2, 3, 0, 4).reshape(32, DLOC, TN)[:NOFF]
        outs.append(core.reshape(NOFF, DLOC, H, W))
    full = np.concatenate(outs, axis=1)  # [27, 64, 64, 64]
    return full[None].astype(np.float32)  # [1, 27, 64, 64, 64]


# revision 18
# speedup vs baseline: 1.0528x; 1.0528x over previous
"""Trainium2 Bass kernel for nn_CorrTorch: 27-shift 3D correlation + 1x1x1 conv.

Math (B=1, C=32, D=H=W=64, NOFF=27):
  cv[(k,c), s] = x1[c,s] * pad(x2)[c, s + off_k] / sqrt(C)    (864 x 64^3)
  out[o, s]    = sum_{k,c} conv_w[o, k*32+c] * cv[(k,c), s] + conv_b[o]

Sharding: D axis split across 8 cores (8 planes each), halo baked into the
per-core x2 slab on the host. No collectives.

Per-core device strategy (one output plane d at a time):
  - x2 slab planes carry FOUR replica groups of 32 channels on 128 partitions;
    group g is pre-shifted (host-baked) by o_g in (dy,dx):
        O = [(0,0), (-2,0), (-1,-1), (-1,1)]
    With a per-instruction 2D slice offset s, one tensor_tensor multiply
    computes the 4 products {o_g + s} at once.  Exhaustive search over all
    4-offset bakes shows 9 multiplies/plane is the floor (27 shifts, <=4 new
    per instruction, and no 4-cell pattern has >3 disjoint in-cube translates);
    this bake achieves it as 2 fulls + 1 half-width partial per dz:
        dz in {0,2}:  full@s=(2,0), full@s=(2,1), groups{0,1}@s=(2,2)
        dz == 1:      full@s=(1,1), full@s=(2,1), groups{2,3}@s=(3,1)
    covering each of the 27 cells exactly once.
  - The two complementary partials of dz=0/dz=1 stack into ONE 128-row cv
    tile, so the 9 products form only EIGHT matmul contraction chunks
    (7x K=128 + 1x K=64) instead of 9 -> tensor-engine time drops ~11%.
  - ~20 of the 72 multiplies run on GPSIMD via scalar_tensor_tensor
    ((x*1)*y), which maps to a faster Q7 codepath than tensor_tensor mult;
    the rest run on the (bottleneck) vector engine in bf16 2x mode.
  - The 1x1 conv is 8 accumulated matmuls per 512-col subtile with M=32
    (27 outputs + 5 zero rows so PSUM is fully written), 4 spatial subtiles
    packed per PSUM bank via tile_position column tiling.
  - ScalarE evicts PSUM -> SBUF bf16 with the conv bias applied, one DMA per
    PSUM tile writes a packed [128, 512] block to HBM; the host un-packs.
"""

import numpy as np
import ml_dtypes

import concourse.bass as bass
import concourse.mybir as mybir
import concourse.tile as tile
from concourse.alu_op_type import AluOpType
from concourse.bass_utils import run_bass_kernel_spmd

C = 32
D = 64
H = 64
W = 64
NOFF = 27
NCORES = 8
DLOC = D // NCORES          # 8 output planes per core
NSLAB = DLOC + 2            # 10 slab planes per core
SROWS = 67                  # baked slab rows per plane (dyv in [0,67))
SCOLS = 66                  # baked slab cols per plane
PLANE_F = SROWS * SCOLS     # 4422 elements per partition per slab plane
TN = H * W                  # 4096 columns per cv tile (one full plane)
SUB = 512                   # columns per matmul (one PSUM bank)
NSUBT = TN // SUB           # 8 spatial subtiles per plane
NCHUNK = 8                  # matmul contraction chunks per plane
M32 = 32                    # matmul output rows per col group (27 + 5 zero)

# Replica-group pre-shift offsets (dy, dx), searched (see module docstring).
OFFS = [(0, 0), (-2, 0), (-1, -1), (-1, 1)]

# Per-plane chunk specs: (dz, s=(sy,sx), partition ranges with group lists).
# Every chunk is one cv tile; chunk 6 holds two half-width instructions.
# cells(g) = OFFS[g] + s must cover {0,1,2}^2 per dz exactly (checked below).
CHUNKS = [
    (0, [((0, 128), (2, 0))]),
    (0, [((0, 128), (2, 1))]),
    (1, [((0, 128), (1, 1))]),
    (1, [((0, 128), (2, 1))]),
    (2, [((0, 128), (2, 0))]),
    (2, [((0, 128), (2, 1))]),
    (None, [((0, 64), (2, 2), 0), ((64, 128), (3, 1), 1)]),  # dz 0 / dz 1
    (2, [((0, 64), (2, 2))]),
]
CHUNK_K = [128, 128, 128, 128, 128, 128, 128, 64]

# Engine balance: DVE does bf16 tensor_tensor at ~0.52 ns/col; GPSIMD does the
# same multiply ~2.7x slower but in parallel.  Per plane, GPSIMD takes chunks
# 7 and 6[0:64] whole plus the tail columns of 6[64:128]; DVE takes the rest.
XSPLIT = 2240  # columns of chunk 6[64:128] on DVE (row-aligned); rest GPSIMD

# Matmul accumulation order: GPSIMD-produced tiles first (they have the
# longest latency but are issued a plane ahead), DVE tiles in issue order.
MM_ORDER = [7, 6, 0, 1, 2, 3, 4, 5]

BF16 = mybir.dt.bfloat16
F32 = mybir.dt.float32

_wsplit_ctr = [0]


def _split_sync_waits(nc, max_waits=1):
    """Walrus in this container accepts at most one sync wait per instruction.
    Hoist excess waits onto NoOp instructions inserted just before, on the
    same engine (same-engine program order preserves the semantics)."""
    for fn in nc.m.functions:
        for bb in fn.blocks:
            new = []
            changed = False
            for ins in bb.instructions:
                si = ins.sync_info
                if si is not None and len(si.on_wait) > max_waits:
                    waits = list(si.on_wait)
                    excess, keep = waits[:-max_waits], waits[-max_waits:]
                    for i in range(0, len(excess), max_waits):
                        _wsplit_ctr[0] += 1
                        new.append(
                            mybir.InstNoOp(
                                name=f"wsplit-{_wsplit_ctr[0]}",
                                engine=ins.engine,
                                sync_info=mybir.SyncInfo(
                                    on_wait=excess[i : i + max_waits], on_update=[]
                                ),
                            )
                        )
                    ins.sync_info = mybir.SyncInfo(
                        on_wait=keep, on_update=list(si.on_update)
                    )
                    changed = True
                new.append(ins)
            if changed:
                bb.instructions = new


def _chunk_cells():
    """Resolve CHUNKS into per-chunk (dz, cell, group) lists and verify the
    27-cell cover is exact."""
    out = []
    seen = set()
    for ci, (dz0, instrs) in enumerate(CHUNKS):
        cells = []
        for spec in instrs:
            (p0, p1), (sy, sx) = spec[0], spec[1]
            dz = dz0 if len(spec) == 2 else spec[2]
            for g in range(p0 // 32, p1 // 32):
                oy, ox = OFFS[g]
                dy, dx = sy + oy, sx + ox
                if 0 <= dy <= 2 and 0 <= dx <= 2:
                    cell = (dz, dy, dx)
                    assert cell not in seen, (ci, cell)
                    seen.add(cell)
                    cells.append((g, dz, dy, dx))
        out.append(cells)
    assert len(seen) == 27, len(seen)
    return out


_CELLS = _chunk_cells()


def build_program():
    nc = bass.Bass()

    x1r = nc.dram_tensor("x1r", [DLOC, 128, TN], BF16, kind="ExternalInput")
    x2r = nc.dram_tensor("x2r", [NSLAB, 128, PLANE_F], BF16, kind="ExternalInput")
    wts = nc.dram_tensor("wts", [128, NCHUNK * M32], BF16, kind="ExternalInput")
    bias = nc.dram_tensor("bias", [128, 1], F32, kind="ExternalInput")
    out = nc.dram_tensor("out", [128, DLOC * 2 * SUB], BF16, kind="ExternalOutput")

    with tile.TileContext(nc) as tc:
        with (
            tc.tile_pool(name="wt", bufs=1) as wt_pool,
            tc.tile_pool(name="x2", bufs=5) as x2_pool,
            tc.tile_pool(name="x1", bufs=3) as x1_pool,
            tc.tile_pool(name="cv", bufs=16) as cv_pool,
            tc.tile_pool(name="stage", bufs=3) as stage_pool,
            tc.tile_pool(name="psum", bufs=6, space="PSUM") as psum_pool,
        ):
            x2t = {}

            def load_x2_plane(p):
                t = x2_pool.tile([128, PLANE_F], BF16, tag="x2plane")
                nc.sync.dma_start(out=t[:], in_=x2r[p])
                x2t[p] = t

            # Pipeline fill: weights first (tiny), then the pieces that the
            # scheduler's first few DVE/PE instructions depend on, smallest
            # dependencies first so compute starts ~6us in.
            wt_tile = wt_pool.tile([128, NCHUNK * M32], BF16)
            nc.sync.dma_start(out=wt_tile[:], in_=wts[:])
            bias_tile = wt_pool.tile([128, 1], F32)
            nc.sync.dma_start(out=bias_tile[:], in_=bias[:])

            t1 = x2_pool.tile([128, PLANE_F], BF16, tag="x2plane")
            nc.sync.dma_start(out=t1[:, 0:2508], in_=x2r[1][:, 0:2508])
            x2t[1] = t1
            x1t0 = x1_pool.tile([128, TN], BF16, tag="x1plane")
            nc.sync.dma_start(out=x1t0[:, 0:2304], in_=x1r[0][:, 0:2304])
            t0 = x2_pool.tile([128, PLANE_F], BF16, tag="x2plane")
            nc.sync.dma_start(out=t0[:, 0:2310], in_=x2r[0][:, 0:2310])
            x2t[0] = t0
            nc.sync.dma_start(out=t0[:, 2310:], in_=x2r[0][:, 2310:])
            nc.sync.dma_start(out=x1t0[:, 2304:], in_=x1r[0][:, 2304:])
            nc.sync.dma_start(out=t1[:, 2508:], in_=x2r[1][:, 2508:])
            load_x2_plane(2)

            pending = None
            for d in range(DLOC):
                if d + 3 < NSLAB:
                    load_x2_plane(d + 3)
                if d == 0:
                    x1t = x1t0
                else:
                    x1t = x1_pool.tile([128, TN], BF16, tag="x1plane")
                    nc.sync.dma_start(out=x1t[:], in_=x1r[d])

                # --- 9 multiplies (split 9.5/2.5-ish DVE/GPSIMD) -> 8 cv tiles
                cvt = []
                for _ci in range(NCHUNK):
                    cv = cv_pool.tile([128, TN], BF16, tag="cv")
                    cvt.append(cv)

                def mult(eng, cv, dz, p0, p1, sy, sx, c0, c1):
                    # c0/c1 must be multiples of W (row-aligned column split)
                    y0, y1 = sy + c0 // W, sy + c1 // W
                    slab = x2t[d + dz][:].rearrange(
                        "p (y x) -> p y x", y=SROWS, x=SCOLS
                    )
                    in1 = slab[p0:p1, y0:y1, sx : sx + W]
                    if eng == "pool":
                        nc.gpsimd.scalar_tensor_tensor(
                            out=cv[p0:p1, c0:c1],
                            in0=x1t[p0:p1, c0:c1],
                            scalar=1.0,
                            in1=in1,
                            op0=AluOpType.mult,
                            op1=AluOpType.mult,
                        )
                    else:
                        nc.vector.tensor_mul(
                            out=cv[p0:p1, c0:c1], in0=x1t[p0:p1, c0:c1], in1=in1
                        )

                def full(ci):
                    dz0, instrs = CHUNKS[ci]
                    (p0, p1), (sy, sx) = instrs[0][0], instrs[0][1]
                    mult("dve", cvt[ci], dz0, p0, p1, sy, sx, 0, TN)

                if d == 0:
                    # Pipeline fill: the scheduler runs the chunk-6 suffix
                    # head first on DVE; its (split) loads land first, and the
                    # first full multiply is halved to follow the partials.
                    mult("pool", cvt[6], 0, 0, 64, 2, 2, 0, TN)
                    mult("pool", cvt[7], 2, 0, 64, 2, 2, 0, TN)
                    mult("pool", cvt[6], 1, 64, 128, 3, 1, XSPLIT, TN)
                    mult("dve", cvt[6], 1, 64, 128, 3, 1, 0, XSPLIT)
                    mult("dve", cvt[0], 0, 0, 128, 2, 0, 0, 2048)
                    mult("dve", cvt[0], 0, 0, 128, 2, 0, 2048, TN)
                    for ci in (1, 2, 3, 4, 5):
                        full(ci)
                else:
                    # GPSIMD: chunk 7, chunk 6 prefix, tail cols of 6 suffix
                    mult("pool", cvt[7], 2, 0, 64, 2, 2, 0, TN)
                    mult("pool", cvt[6], 0, 0, 64, 2, 2, 0, TN)
                    mult("pool", cvt[6], 1, 64, 128, 3, 1, XSPLIT, TN)
                    # DVE: head cols of chunk 6 suffix, then the 6 full chunks
                    mult("dve", cvt[6], 1, 64, 128, 3, 1, 0, XSPLIT)
                    for ci in range(6):
                        full(ci)

                # --- 8 accumulated matmul chunks, 2 PSUM tiles x 4 col groups.
                # Chunks 6/7 (GPSIMD-produced, long latency) are deferred into
                # the NEXT plane's matmul window so the tensor engine always
                # has ready work (stalls reset its p-state ramp).
                psums = []
                for _ph in range(2):
                    ps = psum_pool.tile([128, SUB], F32, tag="ps")
                    psums.append(ps)

                def mm(dd, ps2, cvt2, ci):
                    kp = CHUNK_K[ci]
                    for s in range(NSUBT):
                        nc.tensor.matmul(
                            ps2[s // 4][32 * (s % 4) : 32 * (s % 4) + M32, :],
                            lhsT=wt_tile[0:kp, ci * M32 : (ci + 1) * M32],
                            rhs=cvt2[ci][0:kp, s * SUB : (s + 1) * SUB],
                            start=(ci == 0),
                            stop=(ci == NCHUNK - 1),
                            tile_position=(0, 32 * (s % 4)),
                        )

                def evict(dd, ps2):
                    for half in range(2):
                        stage = stage_pool.tile([128, SUB], BF16, tag="stage")
                        nc.scalar.activation(
                            stage[:],
                            ps2[half][:],
                            mybir.ActivationFunctionType.Identity,
                            bias=bias_tile[:],
                        )
                        col = (2 * dd + half) * SUB
                        nc.sync.dma_start(
                            out=out[:, col : col + SUB], in_=stage[:]
                        )

                mm(d, psums, cvt, 0)
                if pending is not None:
                    pd, pps, pcvt = pending
                    mm(pd, pps, pcvt, 6)
                    mm(pd, pps, pcvt, 7)
                    evict(pd, pps)
                for ci in range(1, 6):
                    mm(d, psums, cvt, ci)
                pending = (d, psums, cvt)

            pd, pps, pcvt = pending
            mm(pd, pps, pcvt, 6)
            mm(pd, pps, pcvt, 7)
            evict(pd, pps)

    _split_sync_waits(nc)
    return nc


_PROGRAM = None


def _get_program():
    global _PROGRAM
    if _PROGRAM is None:
        _PROGRAM = build_program()
    return _PROGRAM


def _prep_inputs(in1, in2, conv_w, conv_b):
    """Build the 8 per-core input maps (bf16 layout prep on host)."""
    x1 = np.ascontiguousarray(np.asarray(in1, np.float32).reshape(C, D, H, W))
    x2 = np.ascontiguousarray(np.asarray(in2, np.float32).reshape(C, D, H, W))
    scale = 1.0 / np.sqrt(np.float32(C))
    Wk = (np.asarray(conv_w, np.float32) * scale).reshape(NOFF, NOFF, C)  # [o,k,c]

    # Weights: [128, 8*32]; row 32g+c, col 32*ci+o = Wk[o, 9dz+3dy+dx, c]
    wts = np.zeros((128, NCHUNK * M32), np.float32)
    for ci, cells in enumerate(_CELLS):
        for (g, dz, dy, dx) in cells:
            k = 9 * dz + 3 * dy + dx
            wts[32 * g : 32 * g + C, ci * M32 : ci * M32 + NOFF] = Wk[:, k, :].T
    wts = wts.astype(ml_dtypes.bfloat16)

    bias128 = np.zeros((128, 1), np.float32)
    cb = np.asarray(conv_b, np.float32)
    for g4 in range(4):
        bias128[32 * g4 : 32 * g4 + NOFF, 0] = cb

    # Globally padded x2 volume: [C, D+2, 66, 66]; plane/row/col = global + 1.
    x2p = np.zeros((C, D + 2, H + 2, W + 2), np.float32)
    x2p[:, 1 : D + 1, 1 : H + 1, 1 : W + 1] = x2

    # Baked slab: buffer_g[dyv, dxv] = x2p_plane[dyv + oy, dxv + ox] (0 outside)
    # so an instruction slice (sy, sx) reads x2p_plane[y + sy + oy, x + sx + ox].
    # Widened scratch plane: rows -2..66 -> 69 (offset +2), cols -1..66 -> 68 (+1).
    in_maps = []
    for m in range(NCORES):
        slab = x2p[:, DLOC * m : DLOC * m + NSLAB]  # [C, 10, 66, 66]
        wide = np.zeros((C, NSLAB, 69, 68), np.float32)
        wide[:, :, 2:68, 1:67] = slab
        x2rep = np.zeros((NSLAB, 128, SROWS, SCOLS), np.float32)
        for g, (oy, ox) in enumerate(OFFS):
            x2rep[:, 32 * g : 32 * g + C] = wide[
                :, :, 2 + oy : 2 + oy + SROWS, 1 + ox : 1 + ox + SCOLS
            ].transpose(1, 0, 2, 3)
        x2rep = x2rep.reshape(NSLAB, 128, PLANE_F).astype(ml_dtypes.bfloat16)

        x1c = x1[:, DLOC * m : DLOC * (m + 1)].reshape(C, DLOC, TN)
        x1rep = (
            np.tile(x1c, (4, 1, 1))
            .reshape(128, DLOC, TN)
            .transpose(1, 0, 2)
            .astype(ml_dtypes.bfloat16)
        )  # [8, 128, 4096]

        in_maps.append(
            {
                "x1r": np.ascontiguousarray(x1rep),
                "x2r": np.ascontiguousarray(x2rep),
                "wts": np.ascontiguousarray(wts),
                "bias": bias128,
            }
        )
    return in_maps


def kernel(in1, in2, conv_w, conv_b):
    nc = _get_program()
    in_maps = _prep_inputs(in1, in2, conv_w, conv_b)
    res = run_bass_kernel_spmd(nc, in_maps, core_ids=list(range(NCORES)))
    outs = []
    for r in res.results:
        # [128, DLOC*2*512] bf16: row 32*g4+o, col (2d+half)*512+c
        a = np.asarray(r["out"], np.float32).reshape(4, 32, DLOC, 2, SUB)
        # -> [o, d, half, g4, c] -> [o, d, 4096]
        core = a.transpose(1, 2, 3, 0, 4).reshape(32, DLOC, TN)[:NOFF]
        outs.append(core.reshape(NOFF, DLOC, H, W))
    full = np.concatenate(outs, axis=1)  # [27, 64, 64, 64]
    return full[None].astype(np.float32)  # [1, 27, 64, 64, 64]
